# revision 108
# baseline (speedup 1.0000x reference)
"""MoE (E=4 experts, top-2 routing) forward on 8 Trainium2 NeuronCores.

Data-parallel over tokens: core i processes batch row i (2048 tokens);
expert weights replicated per core.

Fast path (build_moe_routed_nc): actually routes tokens -- computes the
top-2 assignment on device, compacts tokens into per-expert slot tiles via
prefix-sum/one-hot matmuls (which also emit each slot's combine weight),
gathers slot inputs from a host-supplied bf16 copy of x, and runs each
expert's FFN only on its assigned tokens (capacity 1152 = 9 tiles per
expert) -- ~half the matmul work of the dense approach. There is no
separate combine phase: out (f16) is preloaded with x and every slot
tile's y = w_slot * LN2(...) is indirect-scatter-ADDED (DMA cce add) into
the output rows by token id; empty slots carry w_slot = 0 and add zero to
row 0. Host pre-casts W1/W2 to bf16 and supplies x in bf16/f16, halving
load DMA. Assumes the (verified-on-host) specializations: gate_b=0,
b1=b2=0, g1=g2=1, be1=be2=0, and per-(core,expert) counts <= 1152.

Fallback (build_moe_nc): dense all-experts compute, correct for any
inputs; used when the fast-path preconditions do not hold.
"""

import threading

import numpy as np

import concourse.bass as bass
import concourse.mybir as mybir
import concourse.tile as tile
from concourse import bacc
from concourse.bass import ds, ts
from concourse.masks import make_identity, make_upper_triangular

F32 = mybir.dt.float32
BF16 = mybir.dt.bfloat16
F16 = mybir.dt.float16
I32 = mybir.dt.int32
AF = mybir.ActivationFunctionType
ALU = mybir.AluOpType
AX = mybir.AxisListType

P = 128
D = 1024
E = 4
KC = D // P
NCH = D // 512
LN_EPS = 1e-5
CAP = 1152
TAU = CAP // P  # 9 slot-tiles per expert
N_CORES = 8

def _row1(ap):
    """Lift an AP to have a leading length-1 (partition) dim."""
    return bass.AP(tensor=ap.tensor, offset=ap.offset, ap=[[0, 1]] + list(ap.ap))


def _bcast_rows(ap_row, p=P):
    """Broadcast a [1, N]-ish DRAM AP across p partitions (step-0 partition dim)."""
    inner = [list(d) for d in ap_row.ap if d[1] != 1]
    return bass.AP(tensor=ap_row.tensor, offset=ap_row.offset, ap=[[0, p]] + inner)


def build_moe_nc(T=2048, num_devices=N_CORES):
    TT = T // P
    nc = bacc.Bacc(
        "TRN2", target_bir_lowering=False, debug=False, num_devices=num_devices
    )

    x_d = nc.dram_tensor("x", [T, D], F32, kind="ExternalInput")
    gw_d = nc.dram_tensor("gate_W", [D, E], F32, kind="ExternalInput")
    gb_d = nc.dram_tensor("gate_b", [E], F32, kind="ExternalInput")
    w1_d = nc.dram_tensor("W1", [E, D, D], F32, kind="ExternalInput")
    b1_d = nc.dram_tensor("b1", [E, D], F32, kind="ExternalInput")
    g1_d = nc.dram_tensor("g1", [E, D], F32, kind="ExternalInput")
    be1_d = nc.dram_tensor("be1", [E, D], F32, kind="ExternalInput")
    w2_d = nc.dram_tensor("W2", [E, D, D], F32, kind="ExternalInput")
    b2_d = nc.dram_tensor("b2", [E, D], F32, kind="ExternalInput")
    g2_d = nc.dram_tensor("g2", [E, D], F32, kind="ExternalInput")
    be2_d = nc.dram_tensor("be2", [E, D], F32, kind="ExternalInput")
    out_d = nc.dram_tensor("out", [T, D], F32, kind="ExternalOutput")

    with tile.TileContext(nc) as tc:
        with (
            tc.tile_pool(name="const", bufs=1) as const,
            tc.tile_pool(name="w1p", bufs=12) as w1p,
            tc.tile_pool(name="w2p", bufs=12) as w2p,
            tc.tile_pool(name="repp", bufs=2) as repp,
            tc.tile_pool(name="bvep", bufs=2) as bvep,
            tc.tile_pool(name="accp", bufs=TT) as accp,
            tc.tile_pool(name="workp", bufs=2) as workp,
            tc.tile_pool(name="xinp", bufs=2) as xinp,
            tc.tile_pool(name="statp", bufs=3) as statp,
            tc.tile_pool(name="gstp", bufs=1) as gstp,
        ):
            # ---- constants ----
            id_f32 = const.tile([P, P], F32)
            make_identity(nc, id_f32)
            id_bf16 = const.tile([P, P], BF16)
            make_identity(nc, id_bf16)
            ones_bf = const.tile([1, P], BF16)
            nc.vector.memset(ones_bf, 1.0)
            ones_f32 = const.tile([1, P], F32)
            nc.vector.memset(ones_f32, 1.0)
            eps_sb = const.tile([P, 1], F32)
            nc.vector.memset(eps_sb, LN_EPS)

            gw_sb = const.tile([P, KC, E], F32)
            nc.sync.dma_start(out=gw_sb, in_=gw_d.rearrange("(c p) e -> p c e", p=P))
            gb_sb = const.tile([1, E], F32)
            nc.sync.dma_start(out=gb_sb, in_=_row1(gb_d[:]))

            be2_sb = const.tile([E, D], BF16)
            nc.gpsimd.dma_start(out=be2_sb, in_=be2_d[:, :])  # casting dma

            xt_sb = const.tile([P, KC, T], BF16)  # x^T, matmul lhsT layout
            scores_sb = const.tile([P, TT, E], F32)
            w_sb = const.tile([P, TT, E], F32)
            wT_sb = const.tile([E, TT, P], BF16)

            w1tiles = {}
            w2tiles = {}
            bves = {}

            def load_w_chunk(e, c):
                t1w = w1p.tile([P, D], BF16, tag="w1", name=f"w1_{e}_{c}")
                nc.gpsimd.dma_start(out=t1w, in_=w1_d[e, ts(c, P), :])
                w1tiles[(e, c)] = t1w
                t2w = w2p.tile([P, D], BF16, tag="w2", name=f"w2_{e}_{c}")
                nc.gpsimd.dma_start(out=t2w, in_=w2_d[e, ts(c, P), :])
                w2tiles[(e, c)] = t2w

            for _c in range(KC):
                load_w_chunk(0, _c)

            # ---- prologue: transpose x, gate scores ----
            pre_ctx = tc.tile_pool(name="prep", bufs=2, space="PSUM")
            prep = pre_ctx.__enter__()
            for tt in range(TT):
                xin = xinp.tile([P, D], F32, tag="xin")
                nc.sync.dma_start(out=xin, in_=x_d[ts(tt, P), :])
                tp = prep.tile([P, D], F32, tag="tp")
                for c in range(KC):
                    nc.tensor.transpose(tp[:, ts(c, P)], xin[:, ts(c, P)], id_f32)
                xtg = workp.tile([P, D], F32, tag="n1")
                nc.scalar.copy(out=xtg, in_=tp)
                nc.vector.tensor_copy(
                    out=xt_sb[:, :, ts(tt, P)],
                    in_=tp.rearrange("p (c q) -> p c q", c=KC),
                )
                gps = prep.tile([P, E], F32, tag="gate")
                for c in range(KC):
                    nc.tensor.matmul(
                        gps,
                        xtg[:, ts(c, P)],
                        gw_sb[:, c, :],
                        start=(c == 0),
                        stop=False,
                    )
                nc.tensor.matmul(gps, ones_f32, gb_sb, start=False, stop=True)
                nc.scalar.copy(out=scores_sb[:, tt, :], in_=gps)

            # ---- top-2 softmax over the E=4 scores ----
            s3 = scores_sb  # [P, TT, E]
            m1 = gstp.tile([P, TT], F32, tag="m1")
            nc.vector.tensor_reduce(out=m1, in_=s3, axis=AX.X, op=ALU.max)
            m1b = m1.broadcast_to((P, TT, E))
            eqt = gstp.tile([P, TT, E], F32, tag="eqt")
            nc.vector.tensor_tensor(out=eqt, in0=s3, in1=m1b, op=ALU.is_equal)
            smt = gstp.tile([P, TT, E], F32, tag="smt")
            nc.vector.scalar_tensor_tensor(
                out=smt, in0=eqt, scalar=-1e30, in1=s3, op0=ALU.mult, op1=ALU.add
            )
            m2 = gstp.tile([P, TT], F32, tag="m2")
            nc.vector.tensor_reduce(out=m2, in_=smt, axis=AX.X, op=ALU.max)
            m2b = m2.broadcast_to((P, TT, E))
            ind = gstp.tile([P, TT, E], F32, tag="ind")
            nc.vector.tensor_tensor(out=ind, in0=s3, in1=m2b, op=ALU.is_ge)
            dd = gstp.tile([P, TT, E], F32, tag="dd")
            nc.vector.tensor_tensor(out=dd, in0=s3, in1=m1b, op=ALU.subtract)
            ex = gstp.tile([P, TT, E], F32, tag="ex")
            nc.scalar.activation(out=ex, in_=dd, func=AF.Exp)
            en = gstp.tile([P, TT, E], F32, tag="en")
            nc.vector.tensor_tensor(out=en, in0=ex, in1=ind, op=ALU.mult)
            zs = gstp.tile([P, TT], F32, tag="zs")
            nc.vector.tensor_reduce(out=zs, in_=en, axis=AX.X, op=ALU.add)
            rz = gstp.tile([P, TT], F32, tag="rz")
            nc.vector.reciprocal(out=rz, in_=zs)
            rzb = rz.broadcast_to((P, TT, E))
            nc.vector.tensor_tensor(out=w_sb, in0=en, in1=rzb, op=ALU.mult)
            for tt in range(TT):
                wtp = prep.tile([E, P], F32, tag="gate")
                nc.tensor.transpose(wtp, w_sb[:, tt, :], id_f32)
                nc.scalar.copy(out=wT_sb[:, tt, :], in_=wtp)

            pre_ctx.__exit__(None, None, None)
            zp_ctx = tc.tile_pool(name="zp", bufs=2, space="PSUM")
            zp = zp_ctx.__enter__()
            z2p_ctx = tc.tile_pool(name="z2p", bufs=1, space="PSUM")
            z2p = z2p_ctx.__enter__()
            utp_ctx = tc.tile_pool(name="utp", bufs=2, space="PSUM")
            utp = utp_ctx.__enter__()

            # ---- dense expert loop ----
            acc = {}

            def load_bve(e):
                bve = bvep.tile([1, 2, D], BF16, tag="bve", name=f"bve_{e}")
                nc.gpsimd.dma_start(out=bve[:, 0, :], in_=_row1(b1_d[e, :]))
                nc.gpsimd.dma_start(out=bve[:, 1, :], in_=_row1(b2_d[e, :]))
                bves[e] = bve

            reps = {}

            def load_reps(e):
                g1r = repp.tile([P, D], BF16, tag="g1r", name=f"g1r_{e}")
                nc.gpsimd.dma_start(out=g1r, in_=_bcast_rows(g1_d[e : e + 1, :]))
                be1r = repp.tile([P, D], BF16, tag="be1r", name=f"be1r_{e}")
                nc.gpsimd.dma_start(out=be1r, in_=_bcast_rows(be1_d[e : e + 1, :]))
                g2r = repp.tile([P, D], BF16, tag="g2r", name=f"g2r_{e}")
                nc.gpsimd.dma_start(out=g2r, in_=_bcast_rows(g2_d[e : e + 1, :]))
                reps[e] = (g1r, be1r, g2r)

            PREFETCH = 4  # chunks of expert e+1 issued inside expert e's loop
            for e in range(E):
                if e not in reps:
                    load_reps(e)
                g1r, be1r, g2r = reps[e]
                if e not in bves:
                    load_bve(e)
                for c in range(KC):
                    if (e, c) not in w1tiles:
                        load_w_chunk(e, c)
                w1t = [w1tiles[(e, c)] for c in range(KC)]
                w2t = [w2tiles[(e, c)] for c in range(KC)]
                bve = bves[e]

                for tt in range(TT):
                    if e + 1 < E and TT - PREFETCH - 1 <= tt < TT - 1:
                        pc = tt - (TT - PREFETCH - 1)
                        if (e + 1, pc) not in w1tiles:
                            load_w_chunk(e + 1, pc)
                    if e + 1 < E and tt == TT - 2 and (e + 1) not in reps:
                        load_reps(e + 1)
                    if e + 1 < E and tt == TT - 1 and (e + 1) not in bves:
                        load_bve(e + 1)
                    # --- z = x @ W1 + b1 ---
                    z = zp.tile([P, D], F32, tag="z")
                    for c in range(KC):
                        for n in range(NCH):
                            nc.tensor.matmul(
                                z[:, ds(n * 512, 512)],
                                xt_sb[:, c, ts(tt, P)],
                                w1t[c][:, ds(n * 512, 512)],
                                start=(c == 0),
                                stop=False,
                            )
                    for n in range(NCH):
                        nc.tensor.matmul(
                            z[:, ds(n * 512, 512)],
                            ones_bf,
                            bve[:, 0, ds(n * 512, 512)],
                            start=False,
                            stop=True,
                        )
                    # --- LN1 stats ---
                    st1 = statp.tile([P, 2, 6], F32, tag="st1")
                    nc.vector.bn_stats(out=st1[:, 0, :], in_=z[:, 0:512])
                    nc.vector.bn_stats(out=st1[:, 1, :], in_=z[:, 512:1024])
                    mv1 = statp.tile([P, 2], F32, tag="mv1")
                    nc.vector.bn_aggr(out=mv1, in_=st1)
                    sd1 = statp.tile([P, 1], F32, tag="sd1")
                    nc.scalar.activation(
                        out=sd1, in_=mv1[:, 1:2], func=AF.Sqrt, bias=eps_sb
                    )
                    rs1 = statp.tile([P, 1], F32, tag="rs1")
                    nc.vector.reciprocal(out=rs1, in_=sd1)
                    nmr1 = statp.tile([P, 1], F32, tag="nmr1")
                    nc.vector.tensor_scalar(
                        out=nmr1,
                        in0=mv1[:, 0:1],
                        scalar1=rs1,
                        scalar2=-1.0,
                        op0=ALU.mult,
                        op1=ALU.mult,
                    )
                    # --- u = relu((z - m)*rstd*g1 + be1) ---
                    n1 = workp.tile([P, D], F32, tag="n1")
                    nc.scalar.activation(
                        out=n1, in_=z, func=AF.Identity, bias=nmr1, scale=rs1
                    )
                    nc.vector.tensor_tensor(out=n1, in0=n1, in1=g1r, op=ALU.mult)
                    nc.gpsimd.tensor_tensor(out=n1, in0=n1, in1=be1r, op=ALU.add)
                    u = workp.tile([P, D], BF16, tag="u")
                    nc.scalar.activation(out=u, in_=n1, func=AF.Relu)
                    # --- u^T via PE ---
                    utps = utp.tile([P, D], BF16, tag="utp_bf")
                    for c in range(KC):
                        nc.tensor.transpose(utps[:, ts(c, P)], u[:, ts(c, P)], id_bf16)
                    uT = workp.tile([P, KC, P], BF16, tag="uT")
                    utv = utps.rearrange("p (c q) -> p c q", c=KC)
                    nc.scalar.copy(out=uT[:, 0 : KC // 2, :], in_=utv[:, 0 : KC // 2, :])
                    nc.vector.tensor_copy(
                        out=uT[:, KC // 2 :, :], in_=utv[:, KC // 2 :, :]
                    )
                    # --- z2 = u @ W2 + b2 ---
                    z2 = z2p.tile([P, D], F32, tag="z2")
                    for c in range(KC):
                        for n in range(NCH):
                            nc.tensor.matmul(
                                z2[:, ds(n * 512, 512)],
                                uT[:, c, :],
                                w2t[c][:, ds(n * 512, 512)],
                                start=(c == 0),
                                stop=False,
                            )
                    for n in range(NCH):
                        nc.tensor.matmul(
                            z2[:, ds(n * 512, 512)],
                            ones_bf,
                            bve[:, 1, ds(n * 512, 512)],
                            start=False,
                            stop=True,
                        )
                    # --- LN2 stats ---
                    st2 = statp.tile([P, 2, 6], F32, tag="st2")
                    nc.vector.bn_stats(out=st2[:, 0, :], in_=z2[:, 0:512])
                    nc.vector.bn_stats(out=st2[:, 1, :], in_=z2[:, 512:1024])
                    mv2 = statp.tile([P, 2], F32, tag="mv2")
                    nc.vector.bn_aggr(out=mv2, in_=st2)
                    sd2 = statp.tile([P, 1], F32, tag="sd2")
                    nc.scalar.activation(
                        out=sd2, in_=mv2[:, 1:2], func=AF.Sqrt, bias=eps_sb
                    )
                    rs2 = statp.tile([P, 1], F32, tag="rs2")
                    nc.vector.reciprocal(out=rs2, in_=sd2)
                    rw = statp.tile([P, 1], F32, tag="rw")
                    nc.vector.tensor_scalar_mul(
                        out=rw, in0=rs2, scalar1=w_sb[:, tt, e : e + 1]
                    )
                    nmr2 = statp.tile([P, 1], F32, tag="nmr2")
                    nc.vector.tensor_scalar(
                        out=nmr2,
                        in0=mv2[:, 0:1],
                        scalar1=rw,
                        scalar2=-1.0,
                        op0=ALU.mult,
                        op1=ALU.mult,
                    )
                    # --- y_e = (z2 - m2)*rstd2*w_e*g2 ; acc += y_e ---
                    n2 = workp.tile([P, D], F32, tag="n2")
                    nc.scalar.activation(
                        out=n2, in_=z2, func=AF.Identity, bias=nmr2, scale=rw
                    )
                    nc.vector.tensor_tensor(out=n2, in0=n2, in1=g2r, op=ALU.mult)
                    if e == 0:
                        xres = xinp.tile([P, D], F32, tag="xin")
                        nc.sync.dma_start(out=xres, in_=x_d[ts(tt, P), :])
                        acc[tt] = accp.tile([P, D], F32, tag="acc", name=f"acc_{tt}")
                        nc.gpsimd.tensor_tensor(
                            out=acc[tt], in0=n2, in1=xres, op=ALU.add
                        )
                    else:
                        nc.gpsimd.tensor_tensor(
                            out=acc[tt], in0=n2, in1=acc[tt], op=ALU.add
                        )
            utp_ctx.__exit__(None, None, None)
            z2p_ctx.__exit__(None, None, None)
            zp_ctx.__exit__(None, None, None)
            cpp_ctx = tc.tile_pool(name="cpp", bufs=2, space="PSUM")
            cpp = cpp_ctx.__enter__()

            # ---- finalize phase: out = acc + w @ be2 ----
            for tt in range(TT):
                outt = workp.tile([P, D], F32, tag="n1")
                for n in range(NCH):
                    cps = cpp.tile([P, 512], F32, tag="cp", name=f"cp_{tt}_{n}")
                    nc.tensor.matmul(
                        cps,
                        wT_sb[:, tt, :],
                        be2_sb[:, ds(n * 512, 512)],
                        start=True,
                        stop=True,
                    )
                    nc.vector.tensor_tensor(
                        out=outt[:, ds(n * 512, 512)],
                        in0=cps,
                        in1=acc[tt][:, ds(n * 512, 512)],
                        op=ALU.add,
                    )
                nc.sync.dma_start(out=out_d[ts(tt, P), :], in_=outt)

            cpp_ctx.__exit__(None, None, None)

    nc.compile()
    return nc


def build_moe_routed_nc(T=2048, num_devices=N_CORES):
    """Routed fast path, v2.

    Differences from v1:
      - Host supplies x_bf (bf16 copy of x) and bf16 W1/W2: halves the
        weight/dispatch DMA traffic and removes on-device casts.
      - The combine phase is gone. out (f16) is preloaded with x, each
        expert's slot-tile output y = w_slot * LN(z2) (f16) is indirect-
        scatter-ADDED (cce add) into out rows by token id. Empty slots get
        w_slot = 0 so they add zero to row 0.
      - Per-slot combine weights ride the same one-hot matmuls that build
        the compacted token-id lists (ids and weights side by side in one
        rhs).
      - xg gathers arrive bf16 and are transposed by DMA-transpose (as uT
        already was), freeing the PE of transpose work in the FFN loop.
    """
    TT = T // P  # 16 token tiles
    nc = bacc.Bacc(
        "TRN2", target_bir_lowering=False, debug=False, num_devices=num_devices
    )

    x_d = nc.dram_tensor("x", [T, D], F32, kind="ExternalInput")
    xbf_d = nc.dram_tensor("x_bf", [T, D], BF16, kind="ExternalInput")
    xf16_d = nc.dram_tensor("x_f16", [T, D], F16, kind="ExternalInput")
    gw_d = nc.dram_tensor("gate_W", [D, E], F32, kind="ExternalInput")
    w1_d = nc.dram_tensor("W1", [E, D, D], BF16, kind="ExternalInput")
    w2_d = nc.dram_tensor("W2", [E, D, D], BF16, kind="ExternalInput")
    out_d = nc.dram_tensor("out", [T, D], F16, kind="ExternalOutput")

    with tile.TileContext(nc) as tc:
        with (
            tc.tile_pool(name="const", bufs=1) as const,
            tc.tile_pool(name="w1p", bufs=2) as w1p,
            tc.tile_pool(name="w2p", bufs=2) as w2p,
            tc.tile_pool(name="gstp", bufs=1) as gstp,
            tc.tile_pool(name="xgp", bufs=12) as xgp,
            tc.tile_pool(name="xgtp", bufs=5) as xgtp,
            tc.tile_pool(name="utp", bufs=3) as utp,
            tc.tile_pool(name="workp", bufs=3) as workp,
            tc.tile_pool(name="statp", bufs=6) as statp,
        ):
            # gating-critical constants first (Pool is serial: identity must
            # precede the x-load descriptor generation)
            id_f32 = const.tile([P, P], F32)
            make_identity(nc, id_f32)
            gw_sb = const.tile([P, KC, E], F32)
            nc.sync.dma_start(out=gw_sb, in_=gw_d.rearrange("(c p) e -> p c e", p=P))

            x_res = const.tile([P, TT, D], F32)  # resident x (64KB/part)
            for tt in range(TT):
                nc.gpsimd.dma_start(out=x_res[:, tt, :], in_=x_d[ts(tt, P), :])

            # ---------------- remaining constants ----------------
            id_bf16 = const.tile([P, P], BF16)
            make_identity(nc, id_bf16)
            ones_mat = const.tile([P, P], F32)
            nc.vector.memset(ones_mat, 1.0)
            ltri = const.tile([P, P], F32)  # ltri[p,q] = 1 iff p < q
            make_upper_triangular(nc, ltri, val=1.0, diag=False)
            eps_sb = const.tile([P, 1], F32)
            nc.vector.memset(eps_sb, LN_EPS)

            # iotas
            iota_tok_i = const.tile([P, TT], I32)  # p + 128*tt
            nc.gpsimd.iota(iota_tok_i, pattern=[[P, TT]], base=0, channel_multiplier=1)
            ids_f16 = const.tile([P, TT], F16)
            nc.vector.tensor_copy(out=ids_f16, in_=iota_tok_i)
            iota128_i = const.tile([P, P], I32)  # 0..127 along free, all parts
            nc.gpsimd.iota(iota128_i, pattern=[[1, P]], base=0, channel_multiplier=0)
            iota128_f16 = const.tile([P, P], F16)
            nc.vector.tensor_copy(out=iota128_f16, in_=iota128_i)
            iota4_i = const.tile([P, P, E], I32)  # v[p, q, e] = q
            nc.gpsimd.iota(iota4_i, pattern=[[1, P], [0, E]], base=0, channel_multiplier=0)
            iota4_f16 = const.tile([P, P, E], F16)
            nc.vector.tensor_copy(out=iota4_f16, in_=iota4_i)
            thr_i = const.tile([P, TAU + 1], I32)  # 0,128,...,1152
            nc.gpsimd.iota(thr_i, pattern=[[P, TAU + 1]], base=0, channel_multiplier=0)
            thr_f16 = const.tile([P, TAU + 1], F16)
            nc.vector.tensor_copy(out=thr_f16, in_=thr_i)

            scores_sb = const.tile([P, TT, E], F32)

            # ---------------- gating ----------------
            xtp_ctx = tc.tile_pool(name="xtp", bufs=2)
            xtp = xtp_ctx.__enter__()
            g_ctx = tc.tile_pool(name="gpsum", bufs=2, space="PSUM")
            gp = g_ctx.__enter__()

            xtgs = {}
            s3 = scores_sb
            m1 = gstp.tile([P, TT], F32, tag="m1")
            eqt = gstp.tile([P, TT, E], F32, tag="eqt")
            smt = gstp.tile([P, TT, E], F32, tag="smt")
            m2 = gstp.tile([P, TT], F32, tag="m2")
            ind = gstp.tile([P, TT, E], F32, tag="ind")

            def _bcE(t, off):
                return bass.AP(
                    tensor=t.tensor, offset=t.offset + off,
                    ap=[list(t.ap[0]), [0, E]],
                )

            def gate_mm(tt):
                gps = gp.tile([P, E], F32, tag="gate")
                xtg = xtgs.pop(tt)
                for c in range(KC):
                    nc.tensor.matmul(
                        gps,
                        xtg[:, ts(c, P)],
                        gw_sb[:, c, :],
                        start=(c == 0),
                        stop=(c == KC - 1),
                    )
                nc.scalar.copy(out=scores_sb[:, tt, :], in_=gps)
                # per-tile top-2 masks overlap the DMA-bound gating phase
                nc.vector.tensor_reduce(
                    out=m1[:, tt : tt + 1], in_=s3[:, tt, :], axis=AX.X, op=ALU.max
                )
                nc.vector.tensor_tensor(
                    out=eqt[:, tt, :], in0=s3[:, tt, :], in1=_bcE(m1, tt),
                    op=ALU.is_equal,
                )
                nc.vector.scalar_tensor_tensor(
                    out=smt[:, tt, :], in0=eqt[:, tt, :], scalar=-1e30,
                    in1=s3[:, tt, :], op0=ALU.mult, op1=ALU.add,
                )
                nc.vector.tensor_reduce(
                    out=m2[:, tt : tt + 1], in_=smt[:, tt, :], axis=AX.X, op=ALU.max
                )
                nc.vector.tensor_tensor(
                    out=ind[:, tt, :], in0=s3[:, tt, :], in1=_bcE(m2, tt),
                    op=ALU.is_ge,
                )

            for tt in range(TT):
                tp = gp.tile([P, D], F32, tag="tp")
                for c in range(KC):
                    nc.tensor.transpose(
                        tp[:, ts(c, P)], x_res[:, tt, ts(c, P)], id_f32
                    )
                xtg = xtp.tile([P, D], F32, tag="xt")
                nc.scalar.copy(out=xtg, in_=tp)
                xtgs[tt] = xtg
                if tt > 0:
                    gate_mm(tt - 1)
            gate_mm(TT - 1)
            g_ctx.__exit__(None, None, None)
            xtp_ctx.__exit__(None, None, None)

            # expert weights: plain bf16 loads (host pre-casts), 2 pieces each
            w1tiles = {}
            w2tiles = {}

            def load_w(e, which, pool, tiles, nsplit=4):
                tw = pool.tile([P, KC, D], BF16, tag=which, name=f"{which}_{e}")
                src = (w1_d if which == "w1" else w2_d)[e].rearrange(
                    "(c p) d -> p c d", p=P
                )
                for s in range(nsplit):
                    cs = KC // nsplit
                    nc.gpsimd.dma_start(
                        out=tw[:, s * cs : (s + 1) * cs, :],
                        in_=src[:, s * cs : (s + 1) * cs, :],
                    )
                tiles[e] = tw

            def load_w1(e, nsplit=4):
                load_w(e, "w1", w1p, w1tiles, nsplit)

            def load_w2(e):
                load_w(e, "w2", w2p, w2tiles)

            # ------- softmax weights (top-2 masks computed during gating) ----
            m1b = m1.broadcast_to((P, TT, E))
            dd = gstp.tile([P, TT, E], F32, tag="dd")
            nc.vector.tensor_tensor(out=dd, in0=s3, in1=m1b, op=ALU.subtract)
            ex = gstp.tile([P, TT, E], F32, tag="ex")
            nc.scalar.activation(out=ex, in_=dd, func=AF.Exp)
            en = gstp.tile([P, TT, E], F32, tag="en")
            nc.vector.tensor_tensor(out=en, in0=ex, in1=ind, op=ALU.mult)
            zs = gstp.tile([P, TT], F32, tag="zs")
            nc.vector.tensor_reduce(out=zs, in_=en, axis=AX.X, op=ALU.add)
            rz = gstp.tile([P, TT], F32, tag="rz")
            nc.vector.reciprocal(out=rz, in_=zs)
            rzb = rz.broadcast_to((P, TT, E))
            w_sb = gstp.tile([P, TT, E], F32, tag="w")
            nc.vector.tensor_tensor(out=w_sb, in0=en, in1=rzb, op=ALU.mult)
            w16 = gstp.tile([P, TT, E], F16, tag="w16")
            nc.vector.tensor_copy(out=w16, in_=w_sb)

            load_w1(0, nsplit=2)

            # ---------------- slot assignment (prefix sums) ----------------
            l_ctx = tc.tile_pool(name="lpsum", bufs=1, space="PSUM")
            lp = l_ctx.__enter__()
            posp = lp.tile([P, TT, E], F32, tag="posp")
            cntp = lp.tile([P, E, TT], F32, tag="cntp")
            for tt in range(TT):
                nc.tensor.matmul(
                    posp[:, tt, :], ltri, ind[:, tt, :], start=True, stop=True
                )
                nc.tensor.matmul(
                    cntp[:, :, tt], ones_mat, ind[:, tt, :], start=True, stop=True
                )
            # exclusive prefix over tt per expert: shift-seed then one
            # hardware prefix-scan per expert row
            offa = gstp.tile([P, E, TT], F32, tag="offa")
            nc.vector.memset(offa[:, :, 0:1], 0.0)
            nc.vector.tensor_copy(out=offa[:, :, 1:], in_=cntp[:, :, : TT - 1])
            offb = gstp.tile([P, E, TT], F32, tag="offb")
            for e_ in range(E):
                nc.vector.tensor_tensor_scan(
                    out=offb[:, e_, :],
                    data0=offa[:, e_, :],
                    data1=offa[:, e_, :],
                    initial=0.0,
                    op0=ALU.add,
                    op1=ALU.bypass,
                )
            off = offb  # [P, E, TT] exclusive offsets, replicated over partitions
            lanep_ctx = tc.tile_pool(name="lanep", bufs=2)
            lanep = lanep_ctx.__enter__()
            # full-bank PSUM tiles: each expert's accumulation group gets its
            # own zero region
            listps = [
                lp.tile([P, 512], F32, tag=f"listp{e}", name=f"listp{e}")
                for e in range(E)
            ]
            lists_t = {}  # e -> [P, TAU] i32 token-id list tile
            wslot_t = {}  # e -> [P, TAU] f32 per-slot combine weight tile
            RTG = 4  # token tiles per rt4 build (batches DVE op overhead)

            def assign_chain(e0, e1):
                ew = e1 - e0
                posg = gstp.tile([P, TT, ew], F32, tag=f"posg{e0}")
                off_tte = bass.AP(
                    tensor=off.tensor,
                    offset=off.offset + e0 * TT,
                    ap=[list(off.ap[0]), [1, TT], [TT, ew]],
                )
                nc.vector.tensor_tensor(
                    out=posg, in0=posp[:, :, e0:e1], in1=off_tte, op=ALU.add
                )
                # mask non-selected (token,expert): pos -> CAP (dropped)
                posm = gstp.tile([P, TT, ew], F32, tag=f"posm{e0}")
                nc.vector.scalar_tensor_tensor(
                    out=posm,
                    in0=posg,
                    scalar=-float(CAP),
                    in1=ind[:, :, e0:e1],
                    op0=ALU.add,
                    op1=ALU.mult,
                )
                pos16 = gstp.tile([P, TT, ew], F16, tag=f"pos16{e0}")
                nc.vector.tensor_scalar(
                    out=pos16,
                    in0=posm,
                    scalar1=1.0,
                    scalar2=float(CAP),
                    op0=ALU.mult,
                    op1=ALU.add,
                )
                # tau(p) = #{thr <= pos} ; r = pos - 128*tau ; band masks from
                # adjacent is_ge differences
                geY = gstp.tile([P, TT, ew, TAU + 1], F16, tag=f"geY{e0}")
                thr_b = bass.AP(
                    tensor=thr_f16.tensor,
                    offset=thr_f16.offset,
                    ap=[list(thr_f16.ap[0]), [0, TT], [0, ew], [1, TAU + 1]],
                )
                nc.vector.tensor_tensor(
                    out=geY,
                    in0=pos16.broadcast_to((P, TT, ew, TAU + 1)),
                    in1=thr_b,
                    op=ALU.is_ge,
                )
                taup = gstp.tile([P, TT, ew], F16, tag=f"taup{e0}")
                with nc.allow_low_precision(reason="small exact ints in fp16"):
                    nc.vector.tensor_reduce(
                        out=taup, in_=geY[:, :, :, 1 : TAU + 1], axis=AX.X,
                        op=ALU.add,
                    )
                rp = gstp.tile([P, TT, ew], F16, tag=f"rp{e0}")
                nc.vector.scalar_tensor_tensor(
                    out=rp, in0=taup, scalar=-128.0, in1=pos16,
                    op0=ALU.mult, op1=ALU.add,
                )
                band = gstp.tile([P, TT, ew, TAU], F16, tag=f"band{e0}")
                nc.vector.tensor_tensor(
                    out=band,
                    in0=geY[:, :, :, 0:TAU],
                    in1=geY[:, :, :, 1 : TAU + 1],
                    op=ALU.subtract,
                )
                # idmw[...,0,:] = band * token_id ; idmw[...,1,:] = band * w
                # (the weight half rides the idle gpsimd engine)
                idmw = gstp.tile([P, TT, ew, 2, TAU], F16, tag=f"idmw{e0}")
                ids_b = bass.AP(
                    tensor=ids_f16.tensor,
                    offset=ids_f16.offset,
                    ap=[list(ids_f16.ap[0]), [1, TT], [0, ew], [0, TAU]],
                )
                nc.vector.tensor_tensor(
                    out=idmw[:, :, :, 0, :], in0=band, in1=ids_b, op=ALU.mult
                )
                w16_b = bass.AP(
                    tensor=w16.tensor,
                    offset=w16.offset + e0,
                    ap=[list(w16.ap[0]), [E, TT], [1, ew], [0, TAU]],
                )
                nc.vector.tensor_tensor(
                    out=idmw[:, :, :, 1, :], in0=band, in1=w16_b, op=ALU.mult
                )
                # one-hot scatter matmuls -> token-id lists + weights
                for g in range(TT // RTG):
                    rt4 = lanep.tile([P, RTG, P, ew], F16, tag=f"rt4_{e0}")
                    rp_b = bass.AP(
                        tensor=rp.tensor,
                        offset=rp.offset + g * RTG * ew,
                        ap=[list(rp.ap[0]), [ew, RTG], [0, P], [1, ew]],
                    )
                    iota4_b = bass.AP(
                        tensor=iota4_f16.tensor,
                        offset=iota4_f16.offset,
                        ap=[list(iota4_f16.ap[0]), [0, RTG], [E, P], [1, ew]],
                    )
                    nc.vector.tensor_tensor(
                        out=rt4, in0=rp_b, in1=iota4_b, op=ALU.is_equal
                    )
                    for j in range(RTG):
                        tt = g * RTG + j
                        for le in range(ew):
                            rt_e = bass.AP(
                                tensor=rt4.tensor,
                                offset=rt4.offset + j * P * ew + le,
                                ap=[list(rt4.ap[0]), [ew, P]],
                            )
                            nc.tensor.matmul(
                                listps[e0 + le][:, 0 : 2 * TAU],
                                rt_e,
                                idmw[:, tt, le, :, :],
                                start=(tt == 0),
                                stop=(tt == TT - 1),
                            )
                for le in range(ew):
                    lt = gstp.tile([P, TAU], I32, tag=f"lists{e0 + le}")
                    nc.vector.tensor_copy(out=lt, in_=listps[e0 + le][:, 0:TAU])
                    lists_t[e0 + le] = lt
                    wt = gstp.tile([P, TAU], F32, tag=f"wslot{e0 + le}")
                    nc.vector.tensor_copy(
                        out=wt, in_=listps[e0 + le][:, TAU : 2 * TAU]
                    )
                    wslot_t[e0 + le] = wt

            assign_chain(0, E)
            lanep_ctx.__exit__(None, None, None)
            l_ctx.__exit__(None, None, None)

            # ---------------- expert FFNs (software-pipelined) --------------
            e_ctx = tc.tile_pool(name="epsum", bufs=1, space="PSUM")
            ep = e_ctx.__enter__()
            zp_ctx = tc.tile_pool(name="zp", bufs=2, space="PSUM")
            zp = zp_ctx.__enter__()
            z2p_ctx = tc.tile_pool(name="z2p", bufs=1, space="PSUM")
            z2p = z2p_ctx.__enter__()

            NT = E * TAU
            GPF = 9  # gather prefetch depth
            us = {}
            xgs = {}
            xgTs = {}
            uTs = {}

            def stage_gather(i):
                e, tau = divmod(i, TAU)
                xg = xgp.tile([P, D], BF16, tag="xg")
                nc.gpsimd.indirect_dma_start(
                    out=xg,
                    out_offset=None,
                    in_=xbf_d[:, :],
                    in_offset=bass.IndirectOffsetOnAxis(
                        ap=lists_t[e][:, tau : tau + 1], axis=0
                    ),
                )
                xgs[i] = xg

            def stage_xgt(i):
                xg = xgs.pop(i)
                xtps = ep.tile([P, D], BF16, tag="xgt")
                for c in range(KC):
                    nc.tensor.transpose(xtps[:, ts(c, P)], xg[:, ts(c, P)], id_bf16)
                xgT = xgtp.tile([P, KC, P], BF16, tag="xgT")
                xtv = xtps.rearrange("p (c q) -> p c q", c=KC)
                nc.scalar.copy(out=xgT[:, 0 : KC // 2, :], in_=xtv[:, 0 : KC // 2, :])
                nc.vector.tensor_copy(
                    out=xgT[:, KC // 2 :, :], in_=xtv[:, KC // 2 :, :]
                )
                xgTs[i] = xgT

            def stage_ut(i):
                u = us.pop(i)
                utps = ep.tile([P, D], BF16, tag="ut")
                for c in range(KC):
                    nc.tensor.transpose(utps[:, ts(c, P)], u[:, ts(c, P)], id_bf16)
                uT = utp.tile([P, KC, P], BF16, tag="uT")
                utv = utps.rearrange("p (c q) -> p c q", c=KC)
                nc.scalar.copy(out=uT[:, 0 : KC // 2, :], in_=utv[:, 0 : KC // 2, :])
                nc.vector.tensor_copy(
                    out=uT[:, KC // 2 :, :], in_=utv[:, KC // 2 :, :]
                )
                uTs[i] = uT

            def stage_z(i):
                e, tau = divmod(i, TAU)
                if tau == 0 and e == 0:
                    load_w2(0)
                if tau == 1 and e + 1 < E:
                    load_w1(e + 1)
                if tau == 3 and e + 1 < E:
                    load_w2(e + 1)
                xgT = xgTs.pop(i)
                z = zp.tile([P, D], F32, tag="z")
                w1t = w1tiles[e]
                for c in range(KC):
                    for n in range(2):
                        nc.tensor.matmul(
                            z[:, ds(n * 512, 512)],
                            xgT[:, c, :],
                            w1t[:, c, ds(n * 512, 512)],
                            start=(c == 0),
                            stop=(c == KC - 1),
                        )
                st1 = statp.tile([P, 2, 6], F32, tag="st1")
                nc.vector.bn_stats(out=st1[:, 0, :], in_=z[:, 0:512])
                nc.vector.bn_stats(out=st1[:, 1, :], in_=z[:, 512:1024])
                mv1 = statp.tile([P, 2], F32, tag="mv1")
                nc.vector.bn_aggr(out=mv1, in_=st1)
                sd1 = statp.tile([P, 1], F32, tag="sd1")
                nc.scalar.activation(out=sd1, in_=mv1[:, 1:2], func=AF.Sqrt, bias=eps_sb)
                rs1 = statp.tile([P, 1], F32, tag="rs1")
                nc.vector.reciprocal(out=rs1, in_=sd1)
                nmr1 = statp.tile([P, 1], F32, tag="nmr1")
                nc.vector.tensor_scalar(
                    out=nmr1,
                    in0=mv1[:, 0:1],
                    scalar1=rs1,
                    scalar2=-1.0,
                    op0=ALU.mult,
                    op1=ALU.mult,
                )
                # u = relu((z - m) * rstd)  [g1=1, be1=0]
                u = workp.tile([P, D], BF16, tag="u")
                nc.scalar.activation(out=u, in_=z, func=AF.Relu, bias=nmr1, scale=rs1)
                us[i] = u

            def stage_z2(i):
                e, tau = divmod(i, TAU)
                uT = uTs.pop(i)
                z2 = z2p.tile([P, D], F32, tag="z2")
                w2t = w2tiles[e]
                for c in range(KC):
                    for n in range(2):
                        nc.tensor.matmul(
                            z2[:, ds(n * 512, 512)],
                            uT[:, c, :],
                            w2t[:, c, ds(n * 512, 512)],
                            start=(c == 0),
                            stop=(c == KC - 1),
                        )
                st2 = statp.tile([P, 2, 6], F32, tag="st2")
                nc.vector.bn_stats(out=st2[:, 0, :], in_=z2[:, 0:512])
                nc.vector.bn_stats(out=st2[:, 1, :], in_=z2[:, 512:1024])
                mv2 = statp.tile([P, 2], F32, tag="mv2")
                nc.vector.bn_aggr(out=mv2, in_=st2)
                sd2 = statp.tile([P, 1], F32, tag="sd2")
                nc.scalar.activation(out=sd2, in_=mv2[:, 1:2], func=AF.Sqrt, bias=eps_sb)
                rs2 = statp.tile([P, 1], F32, tag="rs2")
                nc.vector.reciprocal(out=rs2, in_=sd2)
                # y = w_slot * (z2 - m2) * rstd2, scatter-added into out by
                # token id. Empty slots have w_slot = 0 -> add 0 to row 0.
                rw = statp.tile([P, 1], F32, tag="rw")
                nc.vector.tensor_scalar_mul(
                    out=rw, in0=rs2, scalar1=wslot_t[e][:, tau : tau + 1]
                )
                nmr2 = statp.tile([P, 1], F32, tag="nmr2")
                nc.vector.tensor_scalar(
                    out=nmr2,
                    in0=mv2[:, 0:1],
                    scalar1=rw,
                    scalar2=-1.0,
                    op0=ALU.mult,
                    op1=ALU.mult,
                )
                y = workp.tile([P, D], F16, tag="y")
                nc.scalar.activation(
                    out=y, in_=z2, func=AF.Identity, bias=nmr2, scale=rw
                )
                # out AP declares P rows (what one scatter actually moves);
                # the runtime indices address the full [T, D] tensor.
                nc.gpsimd.indirect_dma_start(
                    out=out_d[0:P, :],
                    out_offset=bass.IndirectOffsetOnAxis(
                        ap=lists_t[e][:, tau : tau + 1], axis=0
                    ),
                    in_=y,
                    in_offset=None,
                    compute_op=ALU.add,
                )

            # Warm-up gathers first so the first FFN tiles aren't DMA-gated,
            # then the out<-x preload: one DRAM->DRAM copy on the same Pool
            # queue as the scatters, so it is both sequenced after the warm-up
            # gather transfers and hardware-ordered before scatter(0).
            nc.gpsimd.dma_start(out=out_d[:, :], in_=xf16_d[:, :])
            for i in range(min(GPF, NT)):
                stage_gather(i)
            # p-state warm-up: dep-free dummy transposes keep the PE busy
            # across the lists->gather(0) latency so z(0) starts at full clock
            warm = ep.tile([P, D], BF16, tag="xgt")
            for _w in range(32):
                nc.tensor.transpose(
                    warm[:, ts(_w % KC, P)], id_bf16, id_bf16
                )
            # PE order per iteration: z(i), z2(i-2); the xg transpose runs two
            # iterations ahead of its consumer, u^T immediately after z(i)
            # (two iterations before z2(i) consumes it).
            stage_xgt(0)
            for i in range(NT + 2):
                if i + GPF < NT:
                    stage_gather(i + GPF)
                if i < NT:
                    stage_z(i)
                if 0 <= i - 1 < NT:
                    stage_ut(i - 1)
                if i + 1 < NT:
                    stage_xgt(i + 1)
                if i - 2 >= 0:
                    stage_z2(i - 2)
            z2p_ctx.__exit__(None, None, None)
            zp_ctx.__exit__(None, None, None)
            e_ctx.__exit__(None, None, None)

    nc.compile()
    return nc

_nc_cache = {}
_nc_lock = threading.Lock()


def _get_nc(T, num_devices, kind="routed"):
    key = (T, num_devices, kind)
    with _nc_lock:
        if key not in _nc_cache:
            if kind == "routed":
                _nc_cache[key] = build_moe_routed_nc(T, num_devices)
            else:
                _nc_cache[key] = build_moe_nc(T, num_devices)
        return _nc_cache[key]


def _fast_path_ok(inputs, x):
    """Fast path requires the zero/one parameter specializations and that
    no (core, expert) top-2 assignment count exceeds CAP."""
    try:
        if not (
            np.all(inputs["gate_b"] == 0.0)
            and np.all(inputs["b1"] == 0.0)
            and np.all(inputs["b2"] == 0.0)
            and np.all(inputs["be1"] == 0.0)
            and np.all(inputs["be2"] == 0.0)
            and np.all(inputs["g1"] == 1.0)
            and np.all(inputs["g2"] == 1.0)
        ):
            return False
        gw = np.asarray(inputs["gate_W"], dtype=np.float32)
        scores = x.reshape(-1, x.shape[-1]) @ gw  # [B*N, E]
        B, N, _ = x.shape
        E_ = gw.shape[1]
        order = np.argsort(-scores, axis=-1)[:, :2]
        for b in range(B):
            sel = order[b * N : (b + 1) * N]
            counts = np.bincount(sel.reshape(-1), minlength=E_)
            # margin of 8 guards against host/device fp32 tie-break skew
            if counts.max() > CAP - 8:
                return False
        return True
    except Exception:
        return False


def kernel(**inputs) -> np.ndarray:
    import ml_dtypes

    from concourse.bass_utils import run_bass_kernel_spmd

    x = np.ascontiguousarray(np.asarray(inputs["x"], dtype=np.float32))
    B, N, Dd = x.shape
    assert Dd == D and B == N_CORES, (B, N, Dd)

    if _fast_path_ok(inputs, x):
        nc = _get_nc(N, N_CORES, "routed")
        bf16 = ml_dtypes.bfloat16
        x_bf = np.ascontiguousarray(x.astype(bf16))
        weights = {
            "gate_W": np.ascontiguousarray(
                np.asarray(inputs["gate_W"], dtype=np.float32)
            ),
            "W1": np.ascontiguousarray(
                np.asarray(inputs["W1"], dtype=np.float32).astype(bf16)
            ),
            "W2": np.ascontiguousarray(
                np.asarray(inputs["W2"], dtype=np.float32).astype(bf16)
            ),
        }
        x_f16 = np.ascontiguousarray(x.astype(np.float16))
        in_maps = [
            dict(weights, x=x[i], x_bf=x_bf[i], x_f16=x_f16[i])
            for i in range(N_CORES)
        ]
        res = run_bass_kernel_spmd(nc, in_maps, core_ids=list(range(N_CORES)))
        out = np.stack([r["out"] for r in res.results], axis=0)
        return out.astype(np.float32)
    else:
        nc = _get_nc(N, N_CORES, "dense")
        weights = {
            k: np.ascontiguousarray(np.asarray(inputs[k], dtype=np.float32))
            for k in (
                "gate_W", "gate_b", "W1", "b1", "g1", "be1",
                "W2", "b2", "g2", "be2",
            )
        }
        in_maps = [dict(weights, x=x[i]) for i in range(N_CORES)]
    res = run_bass_kernel_spmd(nc, in_maps, core_ids=list(range(N_CORES)))
    out = np.stack([r["out"] for r in res.results], axis=0)
    return out.astype(np.float32)

